# revision 1
# baseline (speedup 1.0000x reference)
"""Trainium2 Bass kernel for BatchShawMultigraphAttention.

Math (derived from the reference):
  - attn_biases adds a per-row constant to scores -> cancels in softmax.
  - w.sum(-1) == 1 after softmax, so the bias term reduces to "+ biases[e,h]".
  - masked softmax with -1e10 == multiply exp(scores) by binary A (rows are
    never fully masked at 10% density, N=1024).
  So per (b,e,h):
    P = exp(q @ k^T / sqrt(F));  T = A * P
    out = relu( (T @ (v + bias_eh)) / (T @ 1) )

Sharding: 8 cores = (b in 0..3) x (query-row half in 0..1); each core owns
512 softmax rows for all (e,h), reading its A slice exactly once.
Device layout trick: everything is computed transposed ([j, i]) so A can be
host-pre-transposed and streamed contiguously, and the final matmul
T^T-slices (lhsT) produce the output in natural [i, feat] layout directly.
"""

import sys

sys.path.insert(0, "/opt/trn_rl_repo")

import numpy as np
import ml_dtypes

B, E, H, N, F, F_ = 4, 4, 4, 1024, 64, 32
NCORES = 8
IH = N // 2          # 512 query rows per core
JB = N // 128        # 8 key blocks
VA_W = F_ + 1        # v columns + ones column = 33

_compiled = None


def _build():
    import concourse.bass as bass
    import concourse.bacc as bacc
    import concourse.tile as tile
    import concourse.mybir as mybir

    f32 = mybir.dt.float32
    nc = bacc.Bacc("TRN2", target_bir_lowering=False, debug=False,
                   enable_asserts=False, num_devices=NCORES)

    qt_d = nc.dram_tensor("qt", [H, F_, IH], f32, kind="ExternalInput")
    kt_d = nc.dram_tensor("kt", [H, F_, N], f32, kind="ExternalInput")
    va_d = nc.dram_tensor("va", [E, H, JB, 128, VA_W], f32, kind="ExternalInput")
    at_d = nc.dram_tensor("at", [E, JB, 128, IH], f32, kind="ExternalInput")
    out_d = nc.dram_tensor("out", [IH, E * H * F_], f32, kind="ExternalOutput")

    with tile.TileContext(nc) as tc:
        with (
            tc.tile_pool(name="const", bufs=1) as cpool,
            tc.tile_pool(name="pt", bufs=1) as ptpool,
            tc.tile_pool(name="ps", bufs=2, space=bass.MemorySpace.PSUM) as pspool,
            tc.tile_pool(name="po", bufs=6, space=bass.MemorySpace.PSUM) as popool,
            tc.tile_pool(name="eps", bufs=8) as epool,
        ):
            projcm = tc.tile_pool(name="proj", bufs=1)
            projpool = projcm.__enter__()
            kt_t, qt_t = [], []
            for h in range(H):
                kt = projpool.tile([F_, N], f32, tag=f"kt{h}")
                nc.sync.dma_start(kt[:], kt_d[h])
                kt_t.append(kt)
                qt = projpool.tile([F_, IH], f32, tag=f"qt{h}")
                nc.sync.dma_start(qt[:], qt_d[h])
                qt_t.append(qt)
            va_t = cpool.tile([128, E * H * JB * VA_W], f32, tag="va")
            for e in range(E):
                for h in range(H):
                    for jb in range(JB):
                        s = ((e * H + h) * JB + jb) * VA_W
                        nc.sync.dma_start(va_t[:, s:s + VA_W], va_d[e, h, jb])

            # Phase B: P^T[h, jb] = exp(k_jb @ q^T / 8)   [128 j, 512 i]
            pt_t = {}
            for h in range(H):
                for jb in range(JB):
                    st = pspool.tile([128, IH], f32, tag="st")
                    nc.tensor.matmul(st[:], kt_t[h][:, jb * 128:(jb + 1) * 128],
                                     qt_t[h][:], start=True, stop=True)
                    pt = ptpool.tile([128, IH], f32, tag=f"pt{h}_{jb}")
                    nc.scalar.activation(pt[:], st[:],
                                         mybir.ActivationFunctionType.Exp,
                                         scale=float(1.0 / np.sqrt(F_)))
                    pt_t[h, jb] = pt

            projcm.__exit__(None, None, None)
            atcm = tc.tile_pool(name="at", bufs=2)
            atpool = atcm.__enter__()
            ttcm = tc.tile_pool(name="tt", bufs=2)
            ttpool = ttcm.__enter__()
            # Phase C: per e: T^T = A^T * P^T, then out = T @ va (accum over jb).
            # h is processed in groups of 2 so tt slots (16 tags x 2 bufs)
            # recycle mid-iteration, letting DVE muls for the next group/e
            # overlap PE accumulation matmuls for the current one.
            for e in range(E):
                at_t = {}
                for jb in range(JB):
                    at = atpool.tile([128, IH], f32, tag=f"at{jb}")
                    nc.sync.dma_start(at[:], at_d[e, jb])
                    at_t[jb] = at
                for hg in range(H // 2):
                    tt_t = {}
                    for jb in range(JB):
                        for h in (2 * hg, 2 * hg + 1):
                            tt = ttpool.tile([128, IH], f32,
                                             tag=f"tt{h % 2}_{jb}")
                            nc.vector.tensor_mul(tt[:], at_t[jb][:],
                                                 pt_t[h, jb][:])
                            tt_t[h, jb] = tt
                    for h in (2 * hg, 2 * hg + 1):
                        col = (e * H + h) * F_
                        for ib in range(IH // 128):
                            po = popool.tile([128, VA_W], f32, tag="po")
                            for jb in range(JB):
                                s = ((e * H + h) * JB + jb) * VA_W
                                nc.tensor.matmul(
                                    po[:],
                                    tt_t[h, jb][:, ib * 128:(ib + 1) * 128],
                                    va_t[:, s:s + VA_W],
                                    start=(jb == 0), stop=(jb == JB - 1))
                            rec = epool.tile([128, 1], f32, tag="rec")
                            nc.vector.reciprocal(rec[:], po[:, F_:F_ + 1])
                            ot = epool.tile([128, F_], f32, tag="ot")
                            nc.scalar.activation(
                                ot[:], po[:, 0:F_],
                                mybir.ActivationFunctionType.Relu,
                                scale=rec[:])
                            nc.sync.dma_start(
                                out_d[ib * 128:(ib + 1) * 128, col:col + F_],
                                ot[:])
            ttcm.__exit__(None, None, None)
            atcm.__exit__(None, None, None)

    nc.compile()
    return nc


def _prep_core_inputs(b, ih, X, A, kernel_w, biases, aks, akn):
    i0 = ih * IH
    Xb = X[b]                                        # [N, F]
    qt = np.einsum("nf,hfk->hkn", Xb[i0:i0 + IH], aks).astype(np.float32)
    kt = np.einsum("nf,hfk->hkn", Xb, akn).astype(np.float32)
    v = np.einsum("nf,hfk->hnk", Xb, kernel_w)       # [H, N, F_]
    va = np.empty((E, H, JB, 128, VA_W), np.float32)
    for e in range(E):
        for h in range(H):
            vb = v[h] + biases[e, h][None, :]        # [N, F_]
            va[e, h, :, :, :F_] = vb.reshape(JB, 128, F_)
            va[e, h, :, :, F_] = 1.0
    at = np.ascontiguousarray(
        A[b, :, i0:i0 + IH, :].transpose(0, 2, 1)    # [E, N_j, IH]
    ).reshape(E, JB, 128, IH).astype(np.float32)
    return {"qt": qt, "kt": kt, "va": va, "at": at}


def kernel(X, A, kernel, biases, attn_kernel_self, attn_kernel_neighs,
           attn_biases):
    global _compiled
    from concourse import bass_utils

    if _compiled is None:
        _compiled = _build()

    in_maps = [
        _prep_core_inputs(c // 2, c % 2, np.asarray(X), np.asarray(A),
                          np.asarray(kernel), np.asarray(biases),
                          np.asarray(attn_kernel_self),
                          np.asarray(attn_kernel_neighs))
        for c in range(NCORES)
    ]
    res = bass_utils.run_bass_kernel_spmd(_compiled, in_maps,
                                          core_ids=list(range(NCORES)))
    out = np.empty((B, N, E * H * F_), np.float32)
    for c in range(NCORES):
        b, ih = c // 2, c % 2
        out[b, ih * IH:(ih + 1) * IH, :] = res.results[c]["out"]
    return out



# revision 8
# speedup vs baseline: 3.1419x; 3.1419x over previous
"""Trainium2 Bass kernel for BatchShawMultigraphAttention.

Math (derived from the reference):
  - attn_biases adds a per-row constant to scores -> cancels in softmax.
  - w.sum(-1) == 1 after softmax, so the bias term reduces to "+ biases[e,h]".
  - masked softmax with -1e10 == multiply exp(scores) by binary A (rows are
    never fully masked at 10% density, N=1024).
  So per (b,e,h):
    P = exp(q @ k^T / sqrt(F_));  T = A * P
    out = relu( (T @ (v + bias_eh)) / (T @ 1) )

Sharding: 8 cores = (b in 0..3) x (query-row half in 0..1); each core owns
512 softmax rows for all (e,h), reading its A slice exactly once.

Engine plan (per core, cost-model busy):
  - DMA: 11 large transfers (A in bf16 is 4 of them) instead of many small
    ones -- the SP sequencer + HWDGE cost ~650ns per DMA regardless of size.
  - PE: scores as float32r (1 cycle/row at 512 moving), phase-C matmuls in
    bf16 (33 rows each); row-sums accumulate into one shared PSUM tile as
    1-column matmuls so reciprocals batch per edge type.
  - Act: only the 32 exp activations.
  - DVE: mask-multiply A*P in bf16 with all 4 heads fused per op (the A tile
    is broadcast across heads), hitting the 2x dve mode; plus 4 batched
    reciprocals.
  - Pool: normalize+relu via scalar_tensor_tensor (max 0, then * 1/rowsum).
"""

import sys

sys.path.insert(0, "/opt/trn_rl_repo")

import numpy as np
import ml_dtypes

B, E, H, N, F, F_ = 4, 4, 4, 1024, 64, 32
NCORES = 8
IH = N // 2          # 512 query rows per core
JB = N // 128        # 8 key blocks
IB = IH // 128       # 4 query-row blocks
KQ_W = H * N + H * IH      # 6144: kt columns then qt columns
VA_W = E * H * JB * F_ + 1 + 128  # va cols | ones col | 128 zero cols
ONES_COL = E * H * JB * F_  # 4096

_compiled = None


def _build():
    import concourse.bass as bass
    import concourse.bacc as bacc
    import concourse.tile as tile
    import concourse.mybir as mybir

    f32 = mybir.dt.float32
    f32r = mybir.dt.float32r
    bf16 = mybir.dt.bfloat16
    nc = bacc.Bacc("TRN2", target_bir_lowering=False, debug=False,
                   enable_asserts=False, num_devices=NCORES)

    kq_d = nc.dram_tensor("kq", [F_, KQ_W], f32r, kind="ExternalInput")
    va_d = nc.dram_tensor("va", [128, VA_W], bf16, kind="ExternalInput")
    at_d = nc.dram_tensor("at", [E, 128, JB * IH], bf16, kind="ExternalInput")
    out_d = nc.dram_tensor("out", [IH, E * H * F_], f32, kind="ExternalOutput")

    inv_sqrt = float(1.0 / np.sqrt(F_))

    with tile.TileContext(nc) as tc:
        with (
            tc.tile_pool(name="const", bufs=1) as cpool,
            tc.tile_pool(name="at", bufs=2) as atpool,
            tc.tile_pool(name="tt", bufs=2) as ttpool,
            tc.tile_pool(name="st", bufs=2, space=bass.MemorySpace.PSUM) as stpool,
            tc.tile_pool(name="po", bufs=2, space=bass.MemorySpace.PSUM) as popool,
            tc.tile_pool(name="rs", bufs=2, space=bass.MemorySpace.PSUM) as rspool,
            tc.tile_pool(name="eps", bufs=2) as epool,
        ):
            kq = cpool.tile([F_, KQ_W], f32r, tag="kq")
            nc.sync.dma_start(kq[:], kq_d[:])
            va_t = cpool.tile([128, VA_W], bf16, tag="va")
            nc.sync.dma_start(va_t[:], va_d[:])
            at_t = {}
            for e in range(2):
                at_t[e] = atpool.tile([128, JB * IH], bf16, tag=f"at{e % 2}",
                                      name=f"at_{e}")
                nc.sync.dma_start(at_t[e][:], at_d[e])

            # Phase B: P[jb][:, h*IH + i] = exp(k_jb . q_i / sqrt(F_)), bf16
            pt = []
            for jb in range(JB):
                pt.append(cpool.tile([128, H * IH], bf16, tag=f"pt{jb}",
                                     name=f"pt_{jb}"))
            for jb in range(JB):
                for h in range(H):
                    st = stpool.tile([128, IH], f32, tag="st")
                    nc.tensor.matmul(
                        st[:],
                        kq[:, h * N + jb * 128: h * N + (jb + 1) * 128],
                        kq[:, H * N + h * IH: H * N + (h + 1) * IH],
                        start=True, stop=True)
                    nc.scalar.activation(pt[jb][:, h * IH:(h + 1) * IH], st[:],
                                         mybir.ActivationFunctionType.Exp,
                                         scale=inv_sqrt)

            outst = [epool.tile([128, E * H * F_], f32, tag=f"os{ib}",
                             name=f"os_{ib}")
                     for ib in range(IB)]

            # Phase C: per e: T = A (*) P (4 heads fused, A broadcast).
            # One PSUM bank holds all 16 (ib,h) output blocks of this e as
            # column slices (accumulation is per-address, groups interleave),
            # a second holds the 16 row-sum columns; reciprocal batches into
            # one DVE op and normalize+relu into one Pool op per (e, ib).
            for e in range(E):
                if e + 2 < E:
                    at_t[e + 2] = atpool.tile([128, JB * IH], bf16,
                                              tag=f"at{e % 2}",
                                              name=f"at_{e + 2}")
                    nc.sync.dma_start(at_t[e + 2][:], at_d[e + 2])
                rsum = rspool.tile([128, IB * H], f32, tag="rs")
                po = popool.tile([128, IB * H * F_], f32, tag="po")
                # start=True zeroes the whole PSUM bank, so each bank gets
                # exactly one group: a zero-weights matmul clears it, then
                # every block matmul accumulates with start=False.
                zw = va_t[:, VA_W - 128: VA_W]
                nc.tensor.matmul(po[:], zw, va_t[:, 0:IB * H * F_],
                                 start=True, stop=False,
                                 skip_group_check=True)
                nc.tensor.matmul(rsum[:], zw, va_t[:, 0:IB * H],
                                 start=True, stop=False,
                                 skip_group_check=True)
                for jb in range(JB):
                    tt = ttpool.tile([128, H * IH], bf16, tag=f"tt{jb}")
                    nc.vector.tensor_mul(
                        tt[:].rearrange("p (h i) -> p h i", h=H),
                        pt[jb][:].rearrange("p (h i) -> p h i", h=H),
                        at_t[e][:, jb * IH:(jb + 1) * IH]
                        .unsqueeze(1).broadcast_to((128, H, IH)))
                    for ib in range(IB):
                        for h in range(H):
                            lhsT = tt[:, h * IH + ib * 128:
                                      h * IH + (ib + 1) * 128]
                            col = ((e * H + h) * JB + jb) * F_
                            blk = ib * H + h
                            nc.tensor.matmul(
                                po[:, blk * F_:(blk + 1) * F_],
                                lhsT, va_t[:, col:col + F_],
                                start=False, stop=(jb == JB - 1),
                                skip_group_check=True)
                            nc.tensor.matmul(
                                rsum[:, blk: blk + 1],
                                lhsT, va_t[:, ONES_COL: ONES_COL + 1],
                                start=False, stop=(jb == JB - 1),
                                skip_group_check=True)
                rec = epool.tile([128, IB * H], f32, tag="rec")
                nc.vector.reciprocal(rec[:], rsum[:])
                # GPSIMD cannot read PSUM: Act does relu(po) -> SBUF, then
                # Pool multiplies by the per-block broadcast reciprocal.
                postage = epool.tile([128, IB * H * F_], f32, tag="pos")
                nc.scalar.activation(postage[:], po[:],
                                     mybir.ActivationFunctionType.Relu)
                for ib in range(IB):
                    nc.gpsimd.tensor_mul(
                        outst[ib][:, e * H * F_:(e + 1) * H * F_]
                        .rearrange("p (h k) -> p h k", h=H),
                        postage[:, ib * H * F_:(ib + 1) * H * F_]
                        .rearrange("p (h k) -> p h k", h=H),
                        rec[:, ib * H:(ib + 1) * H]
                        .unsqueeze(2).broadcast_to((128, H, F_)))

            for ib in range(IB):
                nc.sync.dma_start(out_d[ib * 128:(ib + 1) * 128, :],
                                  outst[ib][:])

    nc.compile()
    return nc


def _prep_core_inputs(b, ih, X, A, kernel_w, biases, aks, akn):
    i0 = ih * IH
    Xb = X[b]                                        # [N, F]
    kt = np.einsum("nf,hfk->hkn", Xb, akn)           # [H, F_, N]
    qt = np.einsum("nf,hfk->hkn", Xb[i0:i0 + IH], aks)  # [H, F_, IH]
    kq = np.empty((F_, KQ_W), np.float32)
    kq[:, :H * N] = kt.transpose(1, 0, 2).reshape(F_, H * N)
    kq[:, H * N:] = qt.transpose(1, 0, 2).reshape(F_, H * IH)

    v = np.einsum("nf,hfk->hnk", Xb, kernel_w)       # [H, N, F_]
    va = np.empty((128, VA_W), ml_dtypes.bfloat16)
    for e in range(E):
        for h in range(H):
            vb = (v[h] + biases[e, h][None, :]).astype(ml_dtypes.bfloat16)
            c = (e * H + h) * JB * F_
            va[:, c:c + JB * F_] = \
                vb.reshape(JB, 128, F_).transpose(1, 0, 2).reshape(128, JB * F_)
    va[:, ONES_COL] = ml_dtypes.bfloat16(1.0)
    va[:, ONES_COL + 1:] = ml_dtypes.bfloat16(0.0)

    # at[e, p, jb*IH + i] = A[b, e, i0+i, jb*128+p]
    at = np.ascontiguousarray(
        A[b, :, i0:i0 + IH, :].reshape(E, IH, JB, 128).transpose(0, 3, 2, 1)
    ).reshape(E, 128, JB * IH).astype(ml_dtypes.bfloat16)
    return {"kq": kq, "va": va, "at": at}


def kernel(X, A, kernel, biases, attn_kernel_self, attn_kernel_neighs,
           attn_biases):
    global _compiled
    from concourse import bass_utils

    if _compiled is None:
        _compiled = _build()

    X = np.asarray(X, dtype=np.float32)
    A = np.asarray(A, dtype=np.float32)
    kernel = np.asarray(kernel, dtype=np.float32)
    biases = np.asarray(biases, dtype=np.float32)
    aks = np.asarray(attn_kernel_self, dtype=np.float32)
    akn = np.asarray(attn_kernel_neighs, dtype=np.float32)

    in_maps = [
        _prep_core_inputs(c // 2, c % 2, X, A, kernel, biases, aks, akn)
        for c in range(NCORES)
    ]
    res = bass_utils.run_bass_kernel_spmd(_compiled, in_maps,
                                          core_ids=list(range(NCORES)))
    out = np.empty((B, N, E * H * F_), np.float32)
    for c in range(NCORES):
        b, ih = c // 2, c % 2
        out[b, ih * IH:(ih + 1) * IH, :] = res.results[c]["out"]
    return out


# revision 9
# speedup vs baseline: 3.1499x; 1.0025x over previous
"""Trainium2 Bass kernel for BatchShawMultigraphAttention.

Math (derived from the reference):
  - attn_biases adds a per-row constant to scores -> cancels in softmax.
  - w.sum(-1) == 1 after softmax, so the bias term reduces to "+ biases[e,h]".
  - masked softmax with -1e10 == multiply exp(scores) by binary A (rows are
    never fully masked at 10% density, N=1024).
  So per (b,e,h):
    P = exp(q @ k^T / sqrt(F_));  T = A * P
    out = relu( (T @ (v + bias_eh)) / (T @ 1) )

Sharding: 8 cores = (b in 0..3) x (query-row half in 0..1); each core owns
512 softmax rows for all (e,h), reading its A slice exactly once.

Engine plan (per core, cost-model busy):
  - DMA: 11 large transfers (A in bf16 is 4 of them) instead of many small
    ones -- the SP sequencer + HWDGE cost ~650ns per DMA regardless of size.
  - PE: scores as float32r (1 cycle/row at 512 moving), phase-C matmuls in
    bf16 (33 rows each); row-sums accumulate into one shared PSUM tile as
    1-column matmuls so reciprocals batch per edge type.
  - Act: only the 32 exp activations.
  - DVE: mask-multiply A*P in bf16 with all 4 heads fused per op (the A tile
    is broadcast across heads), hitting the 2x dve mode; plus 4 batched
    reciprocals.
  - Pool: normalize+relu via scalar_tensor_tensor (max 0, then * 1/rowsum).
"""

import sys

sys.path.insert(0, "/opt/trn_rl_repo")

import numpy as np
import ml_dtypes

B, E, H, N, F, F_ = 4, 4, 4, 1024, 64, 32
NCORES = 8
IH = N // 2          # 512 query rows per core
JB = N // 128        # 8 key blocks
IB = IH // 128       # 4 query-row blocks
KQ_W = H * N + H * IH      # 6144: kt columns then qt columns
VA_W = E * H * JB * F_ + 1 + 128  # va cols | ones col | 128 zero cols
ONES_COL = E * H * JB * F_  # 4096

_compiled = None


def _build():
    import concourse.bass as bass
    import concourse.bacc as bacc
    import concourse.tile as tile
    import concourse.mybir as mybir

    f32 = mybir.dt.float32
    f32r = mybir.dt.float32r
    bf16 = mybir.dt.bfloat16
    nc = bacc.Bacc("TRN2", target_bir_lowering=False, debug=False,
                   enable_asserts=False, num_devices=NCORES)

    kq_d = nc.dram_tensor("kq", [F_, KQ_W], f32r, kind="ExternalInput")
    va_d = nc.dram_tensor("va", [128, VA_W], bf16, kind="ExternalInput")
    at_d = nc.dram_tensor("at", [E, 128, JB * IH], bf16, kind="ExternalInput")
    out_d = nc.dram_tensor("out", [IH, E * H * F_], f32, kind="ExternalOutput")

    inv_sqrt = float(1.0 / np.sqrt(F_))

    with tile.TileContext(nc) as tc:
        with (
            tc.tile_pool(name="const", bufs=1) as cpool,
            tc.tile_pool(name="at", bufs=2) as atpool,
            tc.tile_pool(name="tt", bufs=2) as ttpool,
            tc.tile_pool(name="st", bufs=2, space=bass.MemorySpace.PSUM) as stpool,
            tc.tile_pool(name="po", bufs=2, space=bass.MemorySpace.PSUM) as popool,
            tc.tile_pool(name="rs", bufs=2, space=bass.MemorySpace.PSUM) as rspool,
            tc.tile_pool(name="eps", bufs=2) as epool,
        ):
            kq = cpool.tile([F_, KQ_W], f32r, tag="kq")
            nc.sync.dma_start(kq[:], kq_d[:])
            at_t = {}
            at_t[0] = atpool.tile([128, JB * IH], bf16, tag="at0",
                                  name="at_0")
            half = JB * IH // 2
            nc.sync.dma_start(at_t[0][:, 0:half], at_d[0, :, 0:half])
            nc.sync.dma_start(at_t[0][:, half:], at_d[0, :, half:])
            va_t = cpool.tile([128, VA_W], bf16, tag="va")
            nc.sync.dma_start(va_t[:], va_d[:])
            at_t[1] = atpool.tile([128, JB * IH], bf16, tag="at1",
                                  name="at_1")
            nc.sync.dma_start(at_t[1][:], at_d[1])

            # Phase B: P[jb][:, h*IH + i] = exp(k_jb . q_i / sqrt(F_)), bf16
            pt = []
            for jb in range(JB):
                pt.append(cpool.tile([128, H * IH], bf16, tag=f"pt{jb}",
                                     name=f"pt_{jb}"))
            for jb in range(JB):
                for h in range(H):
                    st = stpool.tile([128, IH], f32, tag="st")
                    nc.tensor.matmul(
                        st[:],
                        kq[:, h * N + jb * 128: h * N + (jb + 1) * 128],
                        kq[:, H * N + h * IH: H * N + (h + 1) * IH],
                        start=True, stop=True)
                    nc.scalar.activation(pt[jb][:, h * IH:(h + 1) * IH], st[:],
                                         mybir.ActivationFunctionType.Exp,
                                         scale=inv_sqrt)

            outst = [epool.tile([128, E * H * F_], f32, tag=f"os{ib}",
                             name=f"os_{ib}")
                     for ib in range(IB)]

            # Phase C: per e: T = A (*) P (4 heads fused, A broadcast).
            # One PSUM bank holds all 16 (ib,h) output blocks of this e as
            # column slices (accumulation is per-address, groups interleave),
            # a second holds the 16 row-sum columns; reciprocal batches into
            # one DVE op and normalize+relu into one Pool op per (e, ib).
            for e in range(E):
                if e + 2 < E:
                    at_t[e + 2] = atpool.tile([128, JB * IH], bf16,
                                              tag=f"at{e % 2}",
                                              name=f"at_{e + 2}")
                    nc.sync.dma_start(at_t[e + 2][:], at_d[e + 2])
                rsum = rspool.tile([128, IB * H], f32, tag="rs")
                po = popool.tile([128, IB * H * F_], f32, tag="po")
                # start=True zeroes the whole PSUM bank, so each bank gets
                # exactly one group: a zero-weights matmul clears it, then
                # every block matmul accumulates with start=False.
                zw = va_t[:, VA_W - 128: VA_W]
                nc.tensor.matmul(po[:], zw, va_t[:, 0:IB * H * F_],
                                 start=True, stop=False,
                                 skip_group_check=True)
                nc.tensor.matmul(rsum[:], zw, va_t[:, 0:IB * H],
                                 start=True, stop=False,
                                 skip_group_check=True)
                for jb in range(JB):
                    tt = ttpool.tile([128, H * IH], bf16, tag=f"tt{jb}")
                    # Pool takes a few early mask-muls (it idles otherwise);
                    # DVE keeps the rest.
                    eng = nc.gpsimd if (jb == 0 or (e == 0 and jb == 1)) \
                        else nc.vector
                    eng.tensor_mul(
                        tt[:].rearrange("p (h i) -> p h i", h=H),
                        pt[jb][:].rearrange("p (h i) -> p h i", h=H),
                        at_t[e][:, jb * IH:(jb + 1) * IH]
                        .unsqueeze(1).broadcast_to((128, H, IH)))
                    for ib in range(IB):
                        for h in range(H):
                            lhsT = tt[:, h * IH + ib * 128:
                                      h * IH + (ib + 1) * 128]
                            col = ((e * H + h) * JB + jb) * F_
                            blk = ib * H + h
                            nc.tensor.matmul(
                                po[:, blk * F_:(blk + 1) * F_],
                                lhsT, va_t[:, col:col + F_],
                                start=False, stop=(jb == JB - 1),
                                skip_group_check=True)
                            nc.tensor.matmul(
                                rsum[:, blk: blk + 1],
                                lhsT, va_t[:, ONES_COL: ONES_COL + 1],
                                start=False, stop=(jb == JB - 1),
                                skip_group_check=True)
                rec = epool.tile([128, IB * H], f32, tag="rec")
                nc.vector.reciprocal(rec[:], rsum[:])
                # GPSIMD cannot read PSUM: Act does relu(po) -> SBUF, then
                # Pool multiplies by the per-block broadcast reciprocal.
                postage = epool.tile([128, IB * H * F_], f32, tag="pos")
                nc.scalar.activation(postage[:], po[:],
                                     mybir.ActivationFunctionType.Relu)
                for ib in range(IB):
                    nc.gpsimd.tensor_mul(
                        outst[ib][:, e * H * F_:(e + 1) * H * F_]
                        .rearrange("p (h k) -> p h k", h=H),
                        postage[:, ib * H * F_:(ib + 1) * H * F_]
                        .rearrange("p (h k) -> p h k", h=H),
                        rec[:, ib * H:(ib + 1) * H]
                        .unsqueeze(2).broadcast_to((128, H, F_)))
                    nc.sync.dma_start(
                        out_d[ib * 128:(ib + 1) * 128,
                              e * H * F_:(e + 1) * H * F_],
                        outst[ib][:, e * H * F_:(e + 1) * H * F_])

    nc.compile()
    return nc


def _prep_core_inputs(b, ih, X, A, kernel_w, biases, aks, akn):
    i0 = ih * IH
    Xb = X[b]                                        # [N, F]
    kt = np.einsum("nf,hfk->hkn", Xb, akn)           # [H, F_, N]
    qt = np.einsum("nf,hfk->hkn", Xb[i0:i0 + IH], aks)  # [H, F_, IH]
    kq = np.empty((F_, KQ_W), np.float32)
    kq[:, :H * N] = kt.transpose(1, 0, 2).reshape(F_, H * N)
    kq[:, H * N:] = qt.transpose(1, 0, 2).reshape(F_, H * IH)

    v = np.einsum("nf,hfk->hnk", Xb, kernel_w)       # [H, N, F_]
    va = np.empty((128, VA_W), ml_dtypes.bfloat16)
    for e in range(E):
        for h in range(H):
            vb = (v[h] + biases[e, h][None, :]).astype(ml_dtypes.bfloat16)
            c = (e * H + h) * JB * F_
            va[:, c:c + JB * F_] = \
                vb.reshape(JB, 128, F_).transpose(1, 0, 2).reshape(128, JB * F_)
    va[:, ONES_COL] = ml_dtypes.bfloat16(1.0)
    va[:, ONES_COL + 1:] = ml_dtypes.bfloat16(0.0)

    # at[e, p, jb*IH + i] = A[b, e, i0+i, jb*128+p]
    at = np.ascontiguousarray(
        A[b, :, i0:i0 + IH, :].reshape(E, IH, JB, 128).transpose(0, 3, 2, 1)
    ).reshape(E, 128, JB * IH).astype(ml_dtypes.bfloat16)
    return {"kq": kq, "va": va, "at": at}


def kernel(X, A, kernel, biases, attn_kernel_self, attn_kernel_neighs,
           attn_biases):
    global _compiled
    from concourse import bass_utils

    if _compiled is None:
        _compiled = _build()

    X = np.asarray(X, dtype=np.float32)
    A = np.asarray(A, dtype=np.float32)
    kernel = np.asarray(kernel, dtype=np.float32)
    biases = np.asarray(biases, dtype=np.float32)
    aks = np.asarray(attn_kernel_self, dtype=np.float32)
    akn = np.asarray(attn_kernel_neighs, dtype=np.float32)

    in_maps = [
        _prep_core_inputs(c // 2, c % 2, X, A, kernel, biases, aks, akn)
        for c in range(NCORES)
    ]
    res = bass_utils.run_bass_kernel_spmd(_compiled, in_maps,
                                          core_ids=list(range(NCORES)))
    out = np.empty((B, N, E * H * F_), np.float32)
    for c in range(NCORES):
        b, ih = c // 2, c % 2
        out[b, ih * IH:(ih + 1) * IH, :] = res.results[c]["out"]
    return out


# revision 10
# speedup vs baseline: 3.5354x; 1.1224x over previous
"""Trainium2 Bass kernel for BatchShawMultigraphAttention.

Math (derived from the reference):
  - attn_biases adds a per-row constant to scores -> cancels in softmax.
  - w.sum(-1) == 1 after softmax, so the bias term reduces to "+ biases[e,h]".
  - masked softmax with -1e10 == multiply exp(scores) by binary A (rows are
    never fully masked at 10% density, N=1024).
  So per (b,e,h):
    P = exp(q @ k^T / sqrt(F_));  T = A * P
    out = relu( (T @ (v + bias_eh)) / (T @ 1) )

Sharding: 8 cores = (b in 0..3) x (query-row half in 0..1); each core owns
512 softmax rows for all (e,h), reading its A slice exactly once.

Engine plan (per core, cost-model busy):
  - DMA: a few large transfers (SP sequencer + HWDGE cost ~650ns per DMA
    regardless of size), ordered/split so the first mask-mul inputs land
    early: zeros/ones const, per-head k/q chunks, A-slice halves.
  - PE: scores as float32r (1 cycle/row at 512 moving), phase-C matmuls in
    bf16 (32 rows each). start=True wipes a whole PSUM bank, so each bank
    gets one zero-weights matmul to clear it and all block matmuls
    accumulate with start=False (row-sum columns batch in a shared bank).
  - Act: the 32 exp activations + one relu(po) PSUM->SBUF copy per e.
  - DVE: mask-multiply A*P in bf16, all 4 heads fused per op (A broadcast
    across heads, 2x dve mode), in a hand-interleaved (e, jb) order that
    avoids stalling on the exp chain or the A-slice DMAs; plus batched
    reciprocals.
  - Pool: a few mask-muls (it idles otherwise) + most of the normalize
    multiplies (broadcast 1/rowsum); DVE takes the final e's normalize so
    the tail is parallel.
"""

import sys

sys.path.insert(0, "/opt/trn_rl_repo")

import numpy as np
import ml_dtypes

B, E, H, N, F, F_ = 4, 4, 4, 1024, 64, 32
NCORES = 8
IH = N // 2          # 512 query rows per core
JB = N // 128        # 8 key blocks
IB = IH // 128       # 4 query-row blocks
HW = N + IH                 # 1536: one head's kt|qt block in kq
KQ_W = H * HW               # 6144
VA_W = E * H * JB * F_      # 4096
Z_W = 513                   # 512 zero cols + ones col

# (e, jb) mask-muls assigned to Pool; the rest run on DVE in FLAT_ORDER.
POOL_MULS = [(1, 2), (2, 0), (3, 0), (1, 5), (2, 4)]
# DVE order, chosen so each op's inputs (pt[jb] from the exp chain, at[e]
# from the DMA queue) are ready when the engine reaches it, and e3 drains
# last so earlier edge types normalize/store during the stream.
FLAT_ORDER = [
    (0, 0), (1, 0), (0, 1), (1, 1), (0, 2), (0, 3), (1, 3), (0, 4),
    (1, 4), (2, 1), (0, 5), (2, 2), (3, 1), (2, 3), (0, 6), (1, 6),
    (3, 2), (2, 5), (0, 7), (3, 3), (1, 7), (2, 6), (3, 4), (2, 7),
    (3, 5), (3, 6), (3, 7),
]

_compiled = None


def _build():
    import concourse.bass as bass
    import concourse.bacc as bacc
    import concourse.tile as tile
    import concourse.mybir as mybir

    f32 = mybir.dt.float32
    f32r = mybir.dt.float32r
    bf16 = mybir.dt.bfloat16
    nc = bacc.Bacc("TRN2", target_bir_lowering=False, debug=False,
                   enable_asserts=False, num_devices=NCORES)

    kq_d = nc.dram_tensor("kq", [F_, KQ_W], f32r, kind="ExternalInput")
    z_d = nc.dram_tensor("z", [128, Z_W], bf16, kind="ExternalInput")
    va_d = nc.dram_tensor("va", [128, VA_W], bf16, kind="ExternalInput")
    at_d = nc.dram_tensor("at", [E, 128, JB * IH], bf16, kind="ExternalInput")
    out_d = nc.dram_tensor("out", [IH, E * H * F_], f32, kind="ExternalOutput")

    inv_sqrt = float(1.0 / np.sqrt(F_))

    with tile.TileContext(nc) as tc:
        with (
            tc.tile_pool(name="const", bufs=1) as cpool,
            tc.tile_pool(name="at", bufs=1) as atpool,
            tc.tile_pool(name="tt", bufs=2) as ttpool,
            tc.tile_pool(name="st", bufs=2, space=bass.MemorySpace.PSUM) as stpool,
            tc.tile_pool(name="po", bufs=1, space=bass.MemorySpace.PSUM) as popool,
            tc.tile_pool(name="rs", bufs=1, space=bass.MemorySpace.PSUM) as rspool,
            tc.tile_pool(name="eps", bufs=2) as epool,
        ):
            # --- input DMAs, latency-ordered ---
            z_t = cpool.tile([128, Z_W], bf16, tag="z")
            nc.sync.dma_start(z_t[:], z_d[:])
            kq = cpool.tile([F_, KQ_W], f32r, tag="kq")
            for h in range(H):
                nc.sync.dma_start(kq[:, h * HW:(h + 1) * HW],
                                  kq_d[:, h * HW:(h + 1) * HW])
            half = JB * IH // 2
            at_t = {}
            for e in range(E):
                at_t[e] = atpool.tile([128, JB * IH], bf16, tag=f"at{e}",
                                      name=f"at_{e}")
            nc.sync.dma_start(at_t[0][:, 0:half], at_d[0, :, 0:half])
            nc.sync.dma_start(at_t[1][:, 0:half], at_d[1, :, 0:half])
            va_t = cpool.tile([128, VA_W], bf16, tag="va")
            nc.sync.dma_start(va_t[:], va_d[:])
            nc.sync.dma_start(at_t[0][:, half:], at_d[0, :, half:])
            nc.sync.dma_start(at_t[1][:, half:], at_d[1, :, half:])
            nc.sync.dma_start(at_t[2][:], at_d[2])
            nc.sync.dma_start(at_t[3][:], at_d[3])

            # --- phase B: P[jb][:, h*IH+i] = exp(k_jb . q_i / sqrt(F_)) ---
            pt = []
            for jb in range(JB):
                pt.append(cpool.tile([128, H * IH], bf16, tag=f"pt{jb}",
                                     name=f"pt_{jb}"))
            for jb in range(JB):
                for h in range(H):
                    st = stpool.tile([128, IH], f32, tag="st")
                    nc.tensor.matmul(
                        st[:],
                        kq[:, h * HW + jb * 128: h * HW + (jb + 1) * 128],
                        kq[:, h * HW + N: h * HW + N + IH],
                        start=True, stop=True)
                    nc.scalar.activation(pt[jb][:, h * IH:(h + 1) * IH], st[:],
                                         mybir.ActivationFunctionType.Exp,
                                         scale=inv_sqrt)

            outst = [epool.tile([128, E * H * F_], f32, tag=f"os{ib}",
                                name=f"os_{ib}")
                     for ib in range(IB)]

            # --- phase C ---
            # po bank per e (16 blocks of 32 cols); one shared rsum bank for
            # all 4 e (64 one-col sums). One zero-weights matmul per bank
            # (start=True wipes the whole bank), everything else accumulates.
            zw = z_t[:, 0:128]
            po = {}
            for e in range(E):
                po[e] = popool.tile([128, IB * H * F_], f32, tag=f"po{e}",
                                    name=f"po_{e}")
                nc.tensor.matmul(po[e][:], zw, z_t[:, 0:512],
                                 start=True, stop=False, skip_group_check=True)
            rsum = rspool.tile([128, E * IB * H], f32, tag="rs")
            nc.tensor.matmul(rsum[:], zw, z_t[:, 0:E * IB * H],
                             start=True, stop=False, skip_group_check=True)

            jb_done = {e: 0 for e in range(E)}

            def emit_mul(e, jb, eng):
                tt = ttpool.tile([128, H * IH], bf16, tag=f"tt{jb}")
                eng.tensor_mul(
                    tt[:].rearrange("p (h i) -> p h i", h=H),
                    pt[jb][:].rearrange("p (h i) -> p h i", h=H),
                    at_t[e][:, jb * IH:(jb + 1) * IH]
                    .unsqueeze(1).broadcast_to((128, H, IH)))
                jb_done[e] += 1
                last = jb_done[e] == JB
                for ib in range(IB):
                    for h in range(H):
                        lhsT = tt[:, h * IH + ib * 128: h * IH + (ib + 1) * 128]
                        col = ((e * H + h) * JB + jb) * F_
                        blk = ib * H + h
                        nc.tensor.matmul(
                            po[e][:, blk * F_:(blk + 1) * F_],
                            lhsT, va_t[:, col:col + F_],
                            start=False, stop=last,
                            skip_group_check=True)
                        nc.tensor.matmul(
                            rsum[:, e * 16 + blk: e * 16 + blk + 1],
                            lhsT, z_t[:, Z_W - 1: Z_W],
                            start=False, stop=last,
                            skip_group_check=True)

            def emit_norm(e, pool_ibs):
                rec = epool.tile([128, IB * H], f32, tag=f"rec{e % 2}",
                                 name=f"rec_{e}")
                nc.vector.reciprocal(rec[:], rsum[:, e * 16:(e + 1) * 16])
                postage = epool.tile([128, IB * H * F_], f32,
                                     tag=f"pos{e % 2}", name=f"pos_{e}")
                nc.scalar.activation(postage[:], po[e][:],
                                     mybir.ActivationFunctionType.Relu)
                for ib in range(IB):
                    eng = nc.gpsimd if ib in pool_ibs else nc.vector
                    eng.tensor_mul(
                        outst[ib][:, e * H * F_:(e + 1) * H * F_]
                        .rearrange("p (h k) -> p h k", h=H),
                        postage[:, ib * H * F_:(ib + 1) * H * F_]
                        .rearrange("p (h k) -> p h k", h=H),
                        rec[:, ib * H:(ib + 1) * H]
                        .unsqueeze(2).broadcast_to((128, H, F_)))
                    nc.sync.dma_start(
                        out_d[ib * 128:(ib + 1) * 128,
                              e * H * F_:(e + 1) * H * F_],
                        outst[ib][:, e * H * F_:(e + 1) * H * F_])

            # Pool's first mask-mul up front, the rest interleaved into the
            # DVE stream; normalizes emitted right after each e completes.
            pool_left = list(POOL_MULS)
            pe, pjb = pool_left.pop(0)
            emit_mul(pe, pjb, nc.gpsimd)
            done_emitted = set()
            for idx, (e, jb) in enumerate(FLAT_ORDER):
                emit_mul(e, jb, nc.vector)
                if pool_left and idx in (4, 8, 13, 17):
                    pe, pjb = pool_left.pop(0)
                    emit_mul(pe, pjb, nc.gpsimd)
                for ec in range(E):
                    if jb_done[ec] == JB and ec not in done_emitted:
                        done_emitted.add(ec)
                        if ec == 3:
                            emit_norm(ec, pool_ibs=(0, 1))
                        else:
                            emit_norm(ec, pool_ibs=(0, 1, 2, 3))

    nc.compile()
    return nc


def _prep_core_inputs(b, ih, X, A, kernel_w, biases, aks, akn):
    i0 = ih * IH
    Xb = X[b]                                        # [N, F]
    kt = np.einsum("nf,hfk->hkn", Xb, akn)           # [H, F_, N]
    qt = np.einsum("nf,hfk->hkn", Xb[i0:i0 + IH], aks)  # [H, F_, IH]
    kq = np.empty((F_, KQ_W), np.float32)
    for h in range(H):
        kq[:, h * HW: h * HW + N] = kt[h]
        kq[:, h * HW + N: (h + 1) * HW] = qt[h]

    z = np.zeros((128, Z_W), ml_dtypes.bfloat16)
    z[:, Z_W - 1] = ml_dtypes.bfloat16(1.0)

    v = np.einsum("nf,hfk->hnk", Xb, kernel_w)       # [H, N, F_]
    va = np.empty((128, VA_W), ml_dtypes.bfloat16)
    for e in range(E):
        for h in range(H):
            vb = (v[h] + biases[e, h][None, :]).astype(ml_dtypes.bfloat16)
            c = (e * H + h) * JB * F_
            va[:, c:c + JB * F_] = \
                vb.reshape(JB, 128, F_).transpose(1, 0, 2).reshape(128, JB * F_)

    # at[e, p, jb*IH + i] = A[b, e, i0+i, jb*128+p]
    at = np.ascontiguousarray(
        A[b, :, i0:i0 + IH, :].reshape(E, IH, JB, 128).transpose(0, 3, 2, 1)
    ).reshape(E, 128, JB * IH).astype(ml_dtypes.bfloat16)
    return {"kq": kq, "z": z, "va": va, "at": at}


def kernel(X, A, kernel, biases, attn_kernel_self, attn_kernel_neighs,
           attn_biases):
    global _compiled
    from concourse import bass_utils

    if _compiled is None:
        _compiled = _build()

    X = np.asarray(X, dtype=np.float32)
    A = np.asarray(A, dtype=np.float32)
    kernel = np.asarray(kernel, dtype=np.float32)
    biases = np.asarray(biases, dtype=np.float32)
    aks = np.asarray(attn_kernel_self, dtype=np.float32)
    akn = np.asarray(attn_kernel_neighs, dtype=np.float32)

    in_maps = [
        _prep_core_inputs(c // 2, c % 2, X, A, kernel, biases, aks, akn)
        for c in range(NCORES)
    ]
    res = bass_utils.run_bass_kernel_spmd(_compiled, in_maps,
                                          core_ids=list(range(NCORES)))
    out = np.empty((B, N, E * H * F_), np.float32)
    for c in range(NCORES):
        b, ih = c // 2, c % 2
        out[b, ih * IH:(ih + 1) * IH, :] = res.results[c]["out"]
    return out


# revision 11
# speedup vs baseline: 3.6387x; 1.0292x over previous
"""Trainium2 Bass kernel for BatchShawMultigraphAttention.

Math (derived from the reference):
  - attn_biases adds a per-row constant to scores -> cancels in softmax.
  - w.sum(-1) == 1 after softmax, so the bias term reduces to "+ biases[e,h]".
  - masked softmax with -1e10 == multiply exp(scores) by binary A (rows are
    never fully masked at 10% density, N=1024).
  So per (b,e,h):
    P = exp(q @ k^T / sqrt(F_));  T = A * P
    out = relu( (T @ (v + bias_eh)) / (T @ 1) )

Sharding: 8 cores = (b in 0..3) x (query-row half in 0..1); each core owns
512 softmax rows for all (e,h), reading its A slice exactly once.

Engine plan (per core, cost-model busy):
  - DMA: a few large transfers (SP sequencer + HWDGE cost ~650ns per DMA
    regardless of size), ordered/split so the first mask-mul inputs land
    early: zeros/ones const, per-head k/q chunks, A-slice halves.
  - PE: scores as float32r (1 cycle/row at 512 moving), phase-C matmuls in
    bf16 (32 rows each). start=True wipes a whole PSUM bank, so each bank
    gets one zero-weights matmul to clear it and all block matmuls
    accumulate with start=False (row-sum columns batch in a shared bank).
  - Act: the 32 exp activations + one relu(po) PSUM->SBUF copy per e.
  - DVE: mask-multiply A*P in bf16, all 4 heads fused per op (A broadcast
    across heads, 2x dve mode), in a hand-interleaved (e, jb) order that
    avoids stalling on the exp chain or the A-slice DMAs; plus batched
    reciprocals.
  - Pool: a few mask-muls (it idles otherwise) + most of the normalize
    multiplies (broadcast 1/rowsum); DVE takes the final e's normalize so
    the tail is parallel.
"""

import sys

sys.path.insert(0, "/opt/trn_rl_repo")

import numpy as np
import ml_dtypes

B, E, H, N, F, F_ = 4, 4, 4, 1024, 64, 32
NCORES = 8
IH = N // 2          # 512 query rows per core
JB = N // 128        # 8 key blocks
IB = IH // 128       # 4 query-row blocks
HW = N + IH                 # 1536: one head's kt|qt block in kq
KQ_W = H * HW               # 6144
VA_W = E * H * JB * F_      # 4096
Z_W = 513                   # 512 zero cols + ones col

# (e, jb) mask-muls assigned to Pool; the rest run on DVE in FLAT_ORDER.
POOL_MULS = [(1, 2), (2, 0), (3, 0), (1, 5), (2, 4)]
# DVE order, chosen so each op's inputs (pt[jb] from the exp chain, at[e]
# from the DMA queue) are ready when the engine reaches it, and e3 drains
# last so earlier edge types normalize/store during the stream.
FLAT_ORDER = [
    (0, 0), (1, 0), (0, 1), (1, 1), (0, 2), (0, 3), (1, 3), (0, 4),
    (1, 4), (2, 1), (0, 5), (2, 2), (3, 1), (2, 3), (0, 6), (1, 6),
    (3, 2), (2, 5), (0, 7), (3, 3), (1, 7), (2, 6), (3, 4), (2, 7),
    (3, 5), (3, 6), (3, 7),
]

_compiled = None


def _build():
    import concourse.bass as bass
    import concourse.bacc as bacc
    import concourse.tile as tile
    import concourse.mybir as mybir

    f32 = mybir.dt.float32
    f32r = mybir.dt.float32r
    bf16 = mybir.dt.bfloat16
    nc = bacc.Bacc("TRN2", target_bir_lowering=False, debug=False,
                   enable_asserts=False, num_devices=NCORES)

    kq_d = nc.dram_tensor("kq", [F_, KQ_W], f32r, kind="ExternalInput")
    z_d = nc.dram_tensor("z", [128, Z_W], bf16, kind="ExternalInput")
    va_d = nc.dram_tensor("va", [128, VA_W], bf16, kind="ExternalInput")
    at_d = nc.dram_tensor("at", [E, 128, JB * IH], bf16, kind="ExternalInput")
    out_d = nc.dram_tensor("out", [IH, E * H * F_], f32, kind="ExternalOutput")

    inv_sqrt = float(1.0 / np.sqrt(F_))

    with tile.TileContext(nc) as tc:
        with (
            tc.tile_pool(name="const", bufs=1) as cpool,
            tc.tile_pool(name="at", bufs=1) as atpool,
            tc.tile_pool(name="tt", bufs=2) as ttpool,
            tc.tile_pool(name="st", bufs=3, space=bass.MemorySpace.PSUM) as stpool,
            tc.tile_pool(name="po", bufs=1, space=bass.MemorySpace.PSUM) as popool,
            tc.tile_pool(name="rs", bufs=1, space=bass.MemorySpace.PSUM) as rspool,
            tc.tile_pool(name="eps", bufs=2) as epool,
        ):
            # --- input DMAs, latency-ordered ---
            z_t = cpool.tile([128, Z_W], bf16, tag="z")
            nc.sync.dma_start(z_t[:], z_d[:])
            kq = cpool.tile([F_, KQ_W], f32r, tag="kq")
            for h in range(H):
                nc.sync.dma_start(kq[:, h * HW:(h + 1) * HW],
                                  kq_d[:, h * HW:(h + 1) * HW])
            half = JB * IH // 2
            at_t = {}
            for e in range(E):
                at_t[e] = atpool.tile([128, JB * IH], bf16, tag=f"at{e}",
                                      name=f"at_{e}")
            nc.sync.dma_start(at_t[0][:, 0:half], at_d[0, :, 0:half])
            nc.sync.dma_start(at_t[1][:, 0:half], at_d[1, :, 0:half])
            va_t = cpool.tile([128, VA_W], bf16, tag="va")
            nc.sync.dma_start(va_t[:], va_d[:])
            nc.sync.dma_start(at_t[0][:, half:], at_d[0, :, half:])
            nc.sync.dma_start(at_t[1][:, half:], at_d[1, :, half:])
            nc.sync.dma_start(at_t[2][:], at_d[2])
            nc.sync.dma_start(at_t[3][:], at_d[3])

            # --- phase B: P[jb][:, h*IH+i] = exp(k_jb . q_i / sqrt(F_)) ---
            pt = []
            for jb in range(JB):
                pt.append(cpool.tile([128, H * IH], bf16, tag=f"pt{jb}",
                                     name=f"pt_{jb}"))
            pt_emitted = [0]

            def emit_pt(upto):
                while pt_emitted[0] < min(upto, JB):
                    jb = pt_emitted[0]
                    for h in range(H):
                        st = stpool.tile([128, IH], f32, tag="st")
                        nc.tensor.matmul(
                            st[:],
                            kq[:, h * HW + jb * 128: h * HW + (jb + 1) * 128],
                            kq[:, h * HW + N: h * HW + N + IH],
                            start=True, stop=True)
                        nc.scalar.activation(
                            pt[jb][:, h * IH:(h + 1) * IH], st[:],
                            mybir.ActivationFunctionType.Exp,
                            scale=inv_sqrt)
                    pt_emitted[0] += 1

            emit_pt(3)

            outst = epool.tile([128, IB * E * H * F_], f32, tag="os")

            # --- phase C ---
            # po bank per e (16 blocks of 32 cols); one shared rsum bank for
            # all 4 e (64 one-col sums). One zero-weights matmul per bank
            # (start=True wipes the whole bank), everything else accumulates.
            zw = z_t[:, 0:128]
            po = {}
            for e in range(E):
                po[e] = popool.tile([128, IB * H * F_], f32, tag=f"po{e}",
                                    name=f"po_{e}")
                nc.tensor.matmul(po[e][:], zw, z_t[:, 0:512],
                                 start=True, stop=False, skip_group_check=True)
            rsum = rspool.tile([128, E * IB * H], f32, tag="rs")
            nc.tensor.matmul(rsum[:], zw, z_t[:, 0:E * IB * H],
                             start=True, stop=False, skip_group_check=True)

            jb_done = {e: 0 for e in range(E)}

            def emit_mul(e, jb, eng):
                tt = ttpool.tile([128, H * IH], bf16, tag=f"tt{jb}")
                eng.tensor_mul(
                    tt[:].rearrange("p (h i) -> p h i", h=H),
                    pt[jb][:].rearrange("p (h i) -> p h i", h=H),
                    at_t[e][:, jb * IH:(jb + 1) * IH]
                    .unsqueeze(1).broadcast_to((128, H, IH)))
                jb_done[e] += 1
                last = jb_done[e] == JB
                for ib in range(IB):
                    for h in range(H):
                        lhsT = tt[:, h * IH + ib * 128: h * IH + (ib + 1) * 128]
                        col = ((e * H + h) * JB + jb) * F_
                        blk = ib * H + h
                        nc.tensor.matmul(
                            po[e][:, blk * F_:(blk + 1) * F_],
                            lhsT, va_t[:, col:col + F_],
                            start=False, stop=last,
                            skip_group_check=True)
                        nc.tensor.matmul(
                            rsum[:, e * 16 + blk: e * 16 + blk + 1],
                            lhsT, z_t[:, Z_W - 1: Z_W],
                            start=False, stop=last,
                            skip_group_check=True)

            def emit_norm(e, pool_ibs):
                rec = epool.tile([128, IB * H], f32, tag=f"rec{e % 2}",
                                 name=f"rec_{e}")
                nc.vector.reciprocal(rec[:], rsum[:, e * 16:(e + 1) * 16])
                postage = epool.tile([128, IB * H * F_], f32,
                                     tag=f"pos{e % 2}", name=f"pos_{e}")
                nc.scalar.activation(postage[:], po[e][:],
                                     mybir.ActivationFunctionType.Relu)
                for ib in range(IB):
                    eng = nc.gpsimd if ib in pool_ibs else nc.vector
                    c0 = ib * E * H * F_ + e * H * F_
                    eng.tensor_mul(
                        outst[:, c0:c0 + H * F_]
                        .rearrange("p (h k) -> p h k", h=H),
                        postage[:, ib * H * F_:(ib + 1) * H * F_]
                        .rearrange("p (h k) -> p h k", h=H),
                        rec[:, ib * H:(ib + 1) * H]
                        .unsqueeze(2).broadcast_to((128, H, F_)))
                # one DMA for all 4 row blocks of this e: DRAM view
                # [ib, p, col] <- SBUF view [p, ib, col]
                nc.sync.dma_start(
                    out_d[:].rearrange("(ib p) c -> ib p c", ib=IB)
                    [:, :, e * H * F_:(e + 1) * H * F_]
                    .transpose([1, 0, 2]),
                    outst[:].rearrange("p (ib c) -> p ib c", ib=IB)
                    [:, :, e * H * F_:(e + 1) * H * F_])

            # Pool's first mask-mul up front, the rest interleaved into the
            # DVE stream; normalizes emitted right after each e completes.
            pool_left = list(POOL_MULS)
            pe, pjb = pool_left.pop(0)
            emit_mul(pe, pjb, nc.gpsimd)
            done_emitted = set()
            for idx, (e, jb) in enumerate(FLAT_ORDER):
                emit_pt(jb + 3)
                emit_mul(e, jb, nc.vector)
                if pool_left and idx in (4, 8, 13, 17):
                    pe, pjb = pool_left.pop(0)
                    emit_mul(pe, pjb, nc.gpsimd)
                for ec in range(E):
                    if jb_done[ec] == JB and ec not in done_emitted:
                        done_emitted.add(ec)
                        if ec == 3:
                            emit_norm(ec, pool_ibs=(0, 1))
                        else:
                            emit_norm(ec, pool_ibs=(0, 1, 2, 3))

    nc.compile()
    return nc


def _prep_core_inputs(b, ih, X, A, kernel_w, biases, aks, akn):
    i0 = ih * IH
    Xb = X[b]                                        # [N, F]
    kt = np.einsum("nf,hfk->hkn", Xb, akn)           # [H, F_, N]
    qt = np.einsum("nf,hfk->hkn", Xb[i0:i0 + IH], aks)  # [H, F_, IH]
    kq = np.empty((F_, KQ_W), np.float32)
    for h in range(H):
        kq[:, h * HW: h * HW + N] = kt[h]
        kq[:, h * HW + N: (h + 1) * HW] = qt[h]

    z = np.zeros((128, Z_W), ml_dtypes.bfloat16)
    z[:, Z_W - 1] = ml_dtypes.bfloat16(1.0)

    v = np.einsum("nf,hfk->hnk", Xb, kernel_w)       # [H, N, F_]
    va = np.empty((128, VA_W), ml_dtypes.bfloat16)
    for e in range(E):
        for h in range(H):
            vb = (v[h] + biases[e, h][None, :]).astype(ml_dtypes.bfloat16)
            c = (e * H + h) * JB * F_
            va[:, c:c + JB * F_] = \
                vb.reshape(JB, 128, F_).transpose(1, 0, 2).reshape(128, JB * F_)

    # at[e, p, jb*IH + i] = A[b, e, i0+i, jb*128+p]
    at = np.ascontiguousarray(
        A[b, :, i0:i0 + IH, :].reshape(E, IH, JB, 128).transpose(0, 3, 2, 1)
    ).reshape(E, 128, JB * IH).astype(ml_dtypes.bfloat16)
    return {"kq": kq, "z": z, "va": va, "at": at}


def kernel(X, A, kernel, biases, attn_kernel_self, attn_kernel_neighs,
           attn_biases):
    global _compiled
    from concourse import bass_utils

    if _compiled is None:
        _compiled = _build()

    X = np.asarray(X, dtype=np.float32)
    A = np.asarray(A, dtype=np.float32)
    kernel = np.asarray(kernel, dtype=np.float32)
    biases = np.asarray(biases, dtype=np.float32)
    aks = np.asarray(attn_kernel_self, dtype=np.float32)
    akn = np.asarray(attn_kernel_neighs, dtype=np.float32)

    in_maps = [
        _prep_core_inputs(c // 2, c % 2, X, A, kernel, biases, aks, akn)
        for c in range(NCORES)
    ]
    res = bass_utils.run_bass_kernel_spmd(_compiled, in_maps,
                                          core_ids=list(range(NCORES)))
    out = np.empty((B, N, E * H * F_), np.float32)
    for c in range(NCORES):
        b, ih = c // 2, c % 2
        out[b, ih * IH:(ih + 1) * IH, :] = res.results[c]["out"]
    return out
